# revision 19
# baseline (speedup 1.0000x reference)
"""Bass/Trainium2 kernel for nn_CopyGenerator (8-core SPMD).

Sharding: 4-way vocab (tensor parallel) x 2-way rows (data parallel).
Core c = 4*h + q owns rows [2048h, 2048h+2048) and vocab columns
[8000q, 8000q+8000).  The softmax denominator needs a cross-vocab-shard
sum: one AllReduce over 4 ranks per tapered group of row-blocks (GS),
in two independent replica groups ([[0,1,2,3],[4,5,6,7]]) that pipeline
behind compute.  The copy branch stays batch-sharded 8 ways.  A tiny
warmup NEFF with one AllReduce runs first (first collective after boot
pays ~60-75us of channel start latency).

Main-loop engine budget per core (16 row-blocks x 8000 vocab):
  - PE: fp8e4 DoubleRow matmuls (2 fp8 weights/cell, K=256 per pass).
    W is scaled by 32 on host so its values sit in fp8e4's normal range;
    the 1/32 rides the ACT exp scale.  Per 2048-col PSUM chunk: kk0
    stationary -> 4 N<=512 MMs (start), kk1 stationary -> 4 MMs (stop).
    The 256-col LDWEIGHTS (~215ns) hides behind 241ns MMs via the
    background weight buffer.  ~123us PE vs ~266us for bf16.
  - ACT: Exp over [128,2048] PSUM chunks with accum_out row sums
    (~2.0us each, ~128us total) - the critical engine.
  - DVE: per-block scale by m=(1-gate)/S into f16 staging (~40us).
  - DMA: og stores f16 (32MB/core, ~92us spread over rings).
PSUM: one pool of 2x[128,2048]f32 (8 banks); the gate and copy-branch
matmuls borrow slices of the same rotating tiles before the main loop.

Numerics: fp8 h/W give ~1.2% relative error on gen-branch probs; the
output absmax lives in the (untouched, fp16-exact) copy branch, so the
global rel-err stays ~3e-4.  f16 og storage adds only 0.05% (vs 0.4%
bf16).  PAD: host zeroes W[PAD,:], exp(0)=1 is subtracted from the
reduced sum (pad_corr), host zeroes output column PAD.
"""

import os
import sys

for _p in ("/opt/trn_rl_repo", "/root/.axon_site/_ro/trn_rl_repo"):
    if os.path.isdir(_p) and _p not in sys.path:
        sys.path.insert(0, _p)

import numpy as np
import ml_dtypes

import concourse.bacc as bacc
import concourse.tile as tile
from concourse import mybir
from concourse.bass_utils import run_bass_kernel_spmd

# ---------------------------------------------------------------------------
# Problem dimensions (hardcoded per spec)
# ---------------------------------------------------------------------------
B, T, S, V, CV, D = 32, 128, 400, 32000, 600, 512
PAD = 1
NCORES = 8
NQ = 4                    # vocab shards
NH = 2                    # row halves
R = B * T                 # 4096 rows
VS = V // NQ              # 8000 vocab columns per core
RH = R // NH              # 2048 rows per core
RB = 128                  # rows per block (= one batch: T == 128)
NBL = RH // RB            # 16 row blocks per core
# tapered all-reduce groups: the first ARs are delayed ~80us by per-NEFF
# collective-channel init anyway, so front groups are big; the last groups
# are size-1 to shrink the AR+scale+store drain tail
GS = [3, 3, 3, 3, 2, 2]   # sums to NBL
NG = len(GS)
GOFF = [sum(GS[:i]) for i in range(NG)]
GRPOF = []                # block -> (group, index-in-group)
for _g, _n in enumerate(GS):
    for _j in range(_n):
        GRPOF.append((_g, _j))
LB = B // NCORES          # 4 local batches per core (copy branch)
KK = 2                    # 256-deep DoubleRow contraction chunks (2x256=512)
WSC = 32.0                # host-side W scale (fp8 subnormal dodge)
# vocab chunking within a block (PSUM chunk: [128,2048]f32 = 4 banks)
VCH = [2048, 2048, 2048, 1856]  # = 8000
VOFF = [0, 2048, 4096, 6144]
NVC = len(VCH)
# s-dim chunks for the copy branch: 400 = 128+128+128+16
SCH = [128, 128, 128, 16]
SOFF = [0, 128, 256, 384]

F32 = mybir.dt.float32
F16 = mybir.dt.float16
BF16 = mybir.dt.bfloat16
F8 = mybir.dt.float8e4
DR = mybir.MatmulPerfMode.DoubleRow

EXP_BUFS = 28   # in-flight exp tiles ([128,2048] f16) = 7 blocks
OUT_BUFS = 2    # [128, 4000] f16 output staging tiles (2 per block)


def _mm_splits(n):
    """Split a free-dim span into <=512 pieces aligned to 512 (PSUM banks)."""
    out = []
    off = 0
    while off < n:
        w = min(512, n - off)
        out.append((off, w))
        off += w
    return out


def build_program(with_bias: bool, b_copy: float, pad_corr: float):
    # Bacc (not plain Bass): its finalize() runs move_matmul_waits_to_ldweights
    # + generate_event_semaphores, which split multi-sem waits down to the
    # TRN2 limit of one wait per instruction.
    nc = bacc.Bacc()

    hT2 = nc.dram_tensor("hT2", [KK, 128, 2, RH], F8, kind="ExternalInput")
    wT2 = nc.dram_tensor("wT2", [KK, 128, 2, VS], F8, kind="ExternalInput")
    wc8 = nc.dram_tensor("wc8", [D, 1], F8, kind="ExternalInput")
    hTlh = nc.dram_tensor("hTlh", [D, LB * RB], BF16, kind="ExternalInput")
    hTll = nc.dram_tensor("hTll", [D, LB * RB], BF16, kind="ExternalInput")
    wch = nc.dram_tensor("wch", [D, 1], BF16, kind="ExternalInput")
    wcl = nc.dram_tensor("wcl", [D, 1], BF16, kind="ExternalInput")
    attnT = nc.dram_tensor("attnT", [S, LB * RB], F16, kind="ExternalInput")
    smap = nc.dram_tensor("smap", [LB, S, CV], F16, kind="ExternalInput")
    if with_bias:
        ebb = nc.dram_tensor("ebb", [128, VS], F32, kind="ExternalInput")

    og = nc.dram_tensor("og", [RH, VS], F16, kind="ExternalOutput")
    oc = nc.dram_tensor("oc", [LB * RB, CV], F32, kind="ExternalOutput")

    with tile.TileContext(nc) as tc:
        with (
            tc.tile_pool(name="const", bufs=1) as const,
            tc.tile_pool(name="pm", bufs=2, space="PSUM") as pm,
            tc.tile_pool(name="expp", bufs=EXP_BUFS) as expp,
            tc.tile_pool(name="outp", bufs=OUT_BUFS) as outp,
            tc.tile_pool(name="ocp", bufs=2) as ocp,
            tc.tile_pool(name="smapp", bufs=LB * 4) as smapp,
            tc.tile_pool(name="small", bufs=10) as small,
            tc.tile_pool(name="gatep", bufs=NBL + LB) as gatep,
            tc.tile_pool(name="dram", bufs=1, space="DRAM") as dram,
        ):
            # ---------------- prologue: resident loads ----------------
            hT2_t = []
            wT2_t = []
            hTl_t = []
            wc_t = []
            wc8_t = []
            attnT_t = []
            ebb_t = []
            # dummy-AR input staged FIRST on the gpsimd ring so the
            # collective-channel init (~65us, paid on first AR per NEFF)
            # starts at ~1us instead of queueing behind the hT2 load
            dz = small.tile([128, 1], F32, tag="dz", name="dz")
            nc.vector.memset(dz[:], 0.0)
            din = dram.tile([128, 1], F32, tag="din", name="din")
            dout = dram.tile([128, 1], F32, tag="dout", name="dout")
            nc.gpsimd.dma_start(din[:], dz[:])
            nc.gpsimd.collective_compute(
                "AllReduce",
                mybir.AluOpType.add,
                replica_groups=[[0, 1, 2, 3], [4, 5, 6, 7]],
                ins=[din.opt()],
                outs=[dout.opt()],
            )
            # small copy-branch inputs on the sync ring; the copy branch is
            # emitted before the main loop so PE has work during weight loads
            for k in range(D // 128):
                th = const.tile([128, LB * RB], BF16, tag=f"hTlh{k}", name=f"hTlh{k}")
                nc.sync.dma_start(th[:], hTlh[k * 128:(k + 1) * 128, :])
                tl = const.tile([128, LB * RB], BF16, tag=f"hTll{k}", name=f"hTll{k}")
                nc.sync.dma_start(tl[:], hTll[k * 128:(k + 1) * 128, :])
                hTl_t.append((th, tl))
                th = const.tile([128, 1], BF16, tag=f"wch{k}", name=f"wch{k}")
                nc.sync.dma_start(th[:], wch[k * 128:(k + 1) * 128, :])
                tl = const.tile([128, 1], BF16, tag=f"wcl{k}", name=f"wcl{k}")
                nc.sync.dma_start(tl[:], wcl[k * 128:(k + 1) * 128, :])
                wc_t.append((th, tl))
                t8 = const.tile([128, 1], F8, tag=f"wc8{k}", name=f"wc8{k}")
                nc.sync.dma_start(t8[:], wc8[k * 128:(k + 1) * 128, :])
                wc8_t.append(t8)
            for k in range(4):
                sk = SCH[k]
                t = const.tile([128, LB * RB], F16, tag=f"attnT{k}", name=f"attnT{k}")
                nc.sync.dma_start(t[:sk, :], attnT[SOFF[k]:SOFF[k] + sk, :])
                attnT_t.append(t)
            # prefetch ALL smap tiles on the sync ring: streamed on the
            # scalar ring they queue behind the 4MB wT2 shard, and the
            # copy-branch psum tiles that wait on them gate the whole pm-pool
            # rotation (blocks included) until ~30us
            sm_t = [[None] * 4 for _ in range(LB)]
            for l in range(LB):
                for k in range(4):
                    sk = SCH[k]
                    sm = smapp.tile([128, CV], F16, tag="sm", name="sm")
                    nc.sync.dma_start(
                        sm[:sk, :], smap[l, SOFF[k]:SOFF[k] + sk, :]
                    )
                    sm_t[l][k] = sm
            # big resident weights: hT2 on the gpsimd ring, wT2 on the ACT
            # ring, ordered so early blocks' dependencies land first
            for kk in range(KK):
                t = const.tile([128, 2, RH], F8, tag=f"hT2{kk}", name=f"hT2{kk}")
                hT2_t.append(t)
                t = const.tile([128, 2, VS], F8, tag=f"wT2{kk}", name=f"wT2{kk}")
                wT2_t.append(t)
            for kk in range(KK):
                nc.gpsimd.dma_start(hT2_t[kk][:], hT2[kk])
            for i in range(NVC):
                vo, n = VOFF[i], VCH[i]
                for kk in range(KK):
                    nc.scalar.dma_start(
                        wT2_t[kk][:, :, vo:vo + n], wT2[kk, :, :, vo:vo + n]
                    )
            if with_bias:
                for i in range(NVC):
                    t = const.tile([128, VCH[i]], F32, tag=f"ebb{i}", name=f"ebb{i}")
                    nc.sync.dma_start(t[:], ebb[:, VOFF[i]:VOFF[i] + VCH[i]])
                    ebb_t.append(t)

            # ---------------- main loop ----------------
            exp_tiles = [[None] * NVC for _ in range(NBL)]
            e_tiles = [None] * NBL    # exp(-gate logit) per block [128,1] f32
            u_tiles = [None] * NBL    # 1 + e per block [128,1] f32
            sg_tiles = [None] * NG    # group local sums [128, GROUP]
            cc_out = [None] * NG      # group all-reduced sums (SBUF)

            def compute_gates_all():
                # all 16 gates into ONE psum tile (one column per block):
                # separate [128,1] psum tiles would ping-pong the 2-buf pool
                # against ACT with a ~1us sem round-trip per gate (~13us of
                # PE idle); here the 64 tiny MMs run back-to-back and a
                # single ACT exp covers every gate
                gps_all = pm.tile([128, 2048], F32, tag="pm", name="gps_all")
                for jb in range(NBL):
                    cb = slice(jb * RB, (jb + 1) * RB)
                    kc = 0
                    for kk in range(KK):
                        for i in range(2):
                            nc.tensor.matmul(
                                gps_all[:, jb:jb + 1],
                                hT2_t[kk][:, i, cb], wc8_t[kc][:],
                                start=(kc == 0), stop=(kc == 3),
                            )
                            kc += 1
                # sigmoid via the Exp table: e = exp(-(x/WSC + b_copy))
                e_all = gatep.tile([128, NBL], F32, tag="e", name="e", bufs=1)
                nc.scalar.activation(
                    e_all[:], gps_all[:, :NBL],
                    mybir.ActivationFunctionType.Exp,
                    bias=-float(b_copy), scale=-1.0 / WSC,
                )
                u_all = gatep.tile([128, NBL], F32, tag="u", name="u", bufs=1)
                nc.vector.tensor_scalar_add(u_all[:], e_all[:], 1.0)
                for jb in range(NBL):
                    e_tiles[jb] = e_all[:, jb:jb + 1]
                    u_tiles[jb] = u_all[:, jb:jb + 1]

            def compute_block(jb):
                cb = slice(jb * RB, (jb + 1) * RB)
                sp = small.tile([128, NVC], F32, tag="sp", name="sp")
                for i in range(NVC):
                    n = VCH[i]
                    ps = pm.tile([128, 2048], F32, tag="pm", name="pm")
                    for kk in range(KK):
                        for (o, w) in _mm_splits(n):
                            nc.tensor.matmul(
                                ps[:, o:o + w],
                                hT2_t[kk][:, :, cb],
                                wT2_t[kk][:, :, VOFF[i] + o:VOFF[i] + o + w],
                                start=(kk == 0), stop=(kk == KK - 1),
                                perf_mode=DR,
                            )
                    ex = expp.tile([128, 2048], F16, tag="exp", name="exp")
                    if not with_bias:
                        nc.scalar.activation(
                            ex[:, :n], ps[:, :n],
                            mybir.ActivationFunctionType.Exp,
                            scale=1.0 / WSC,
                            accum_out=sp[:, i:i + 1],
                        )
                    else:
                        nc.scalar.activation(
                            ex[:, :n], ps[:, :n],
                            mybir.ActivationFunctionType.Exp,
                            scale=1.0 / WSC,
                        )
                        nc.vector.tensor_tensor(
                            ex[:, :n], ex[:, :n], ebb_t[i][:, :n],
                            mybir.AluOpType.mult,
                        )
                        nc.vector.reduce_sum(
                            sp[:, i:i + 1], ex[:, :n],
                            axis=mybir.AxisListType.X,
                        )
                    exp_tiles[jb][i] = ex
                g, j = GRPOF[jb]
                nc.vector.reduce_sum(
                    sg_tiles[g][:, j:j + 1], sp[:], axis=mybir.AxisListType.X
                )

            def scale_block(jb):
                g, j = GRPOF[jb]
                sgl = cc_out[g]
                # m = (1-gate)/S = e / ((1+e) * (S_allreduce - pad_corr))
                corr = small.tile([128, 1], F32, tag="corr", name="corr")
                nc.vector.tensor_scalar_add(corr[:], sgl[:, j:j + 1], -pad_corr)
                v = small.tile([128, 1], F32, tag="v", name="v")
                nc.vector.tensor_scalar(
                    v[:], corr[:], u_tiles[jb][:], None, mybir.AluOpType.mult
                )
                rec = small.tile([128, 1], F32, tag="rec", name="rec")
                nc.vector.reciprocal(rec[:], v[:])
                m = small.tile([128, 1], F32, tag="m", name="m")
                nc.vector.tensor_scalar(
                    m[:], rec[:], e_tiles[jb][:], None, mybir.AluOpType.mult
                )
                # scale exp chunks into f16 staging tiles, 2 stores per block
                for half in range(2):
                    hn = 4096 if half == 0 else VS - 4096
                    ot = outp.tile([128, 4096], F16, tag="ot", name="ot")
                    for i in range(2 * half, 2 * half + 2):
                        n = VCH[i]
                        oo = VOFF[i] - 4096 * half
                        nc.vector.tensor_scalar(
                            ot[:, oo:oo + n],
                            exp_tiles[jb][i][:, :n], m[:], None,
                            mybir.AluOpType.mult,
                        )
                        exp_tiles[jb][i] = None
                    nc.sync.dma_start(
                        og[jb * RB:(jb + 1) * RB, 4096 * half:4096 * half + hn],
                        ot[:, :hn],
                    )

            # ---------------- copy branch (batch-sharded) ----------------
            def emit_copy_branch():
                # emitted first: no dependence on collectives or the big weights
                # local gates: fp32-grade dot via bf16 hi/lo split, all LB
                # gates into one psum tile (see compute_gates_all)
                gpsc = pm.tile([128, 2048], F32, tag="pm", name="gpsc")
                nmm = 3 * (D // 128)
                for l in range(LB):
                    tb = slice(l * RB, (l + 1) * RB)
                    imm = 0
                    for k in range(D // 128):
                        for (a, b_) in ((0, 0), (0, 1), (1, 0)):
                            nc.tensor.matmul(
                                gpsc[:, l:l + 1], hTl_t[k][a][:, tb],
                                wc_t[k][b_][:],
                                start=(imm == 0), stop=(imm == nmm - 1),
                            )
                            imm += 1
                el = gatep.tile([128, LB], F32, tag="el", name="el", bufs=1)
                nc.scalar.activation(
                    el[:], gpsc[:, :LB], mybir.ActivationFunctionType.Exp,
                    bias=-float(b_copy), scale=-1.0,
                )
                ul = gatep.tile([128, LB], F32, tag="ul", name="ul", bufs=1)
                nc.vector.tensor_scalar_add(ul[:], el[:], 1.0)
                gl = gatep.tile([128, LB], F32, tag="gl", name="gl", bufs=1)
                nc.vector.reciprocal(gl[:], ul[:])
                for l in range(LB):
                    tb = slice(l * RB, (l + 1) * RB)
                    cpsf = pm.tile([128, 2048], F32, tag="pm", name="cpsf")
                    cps = cpsf[:, :CV]
                    for k in range(4):
                        sk = SCH[k]
                        sm = sm_t[l][k]
                        for (o, w) in _mm_splits(CV):
                            nc.tensor.matmul(
                                cps[:, o:o + w],
                                attnT_t[k][:sk, tb],
                                sm[:sk, o:o + w],
                                start=(k == 0), stop=(k == 3),
                            )
                    oct_ = ocp.tile([128, CV], F32, tag="oct", name="oct")
                    nc.vector.tensor_scalar(
                        oct_[:], cps, gl[:, l:l + 1], None, mybir.AluOpType.mult
                    )
                    nc.sync.dma_start(oc[tb, :], oct_[:])

            # gates first: they only need hT2 + wc8 (land ~7us), so the PE
            # starts ~8us in; the copy branch follows while wT2 streams
            compute_gates_all()
            emit_copy_branch()

            for g in range(NG):
                gn = GS[g]
                sg_tiles[g] = small.tile([128, gn], F32, tag="sg", name="sg")
                for j in range(gn):
                    compute_block(GOFF[g] + j)
                # all-reduce this group's local sums across the 4 vocab shards
                cin = dram.tile([128, gn], F32, tag=f"cin{g}", name=f"cin{g}")
                cout = dram.tile([128, gn], F32, tag=f"cout{g}", name=f"cout{g}")
                nc.gpsimd.dma_start(cin[:], sg_tiles[g][:])
                nc.gpsimd.collective_compute(
                    "AllReduce",
                    mybir.AluOpType.add,
                    replica_groups=[[0, 1, 2, 3], [4, 5, 6, 7]],
                    ins=[cin.opt()],
                    outs=[cout.opt()],
                )
                sgl = small.tile([128, gn], F32, tag="sgl", name="sgl")
                nc.gpsimd.dma_start(sgl[:], cout[:])
                cc_out[g] = sgl
                for j in range(gn):
                    scale_block(GOFF[g] + j)

    nc.finalize()
    return nc


_warmed_up = False


def _warmup_collectives():
    """Run a minimal NEFF with one AllReduce so the collective channel
    (ncfw firmware / TOPSP) is warm before the main kernel executes."""
    global _warmed_up
    if _warmed_up:
        return
    nc = bacc.Bacc()
    x = nc.dram_tensor("x", [128, 4], F32, kind="ExternalInput")
    y = nc.dram_tensor("y", [128, 4], F32, kind="ExternalOutput")
    with tile.TileContext(nc) as tc:
        with (
            tc.tile_pool(name="sb", bufs=2) as sb,
            tc.tile_pool(name="dr", bufs=2, space="DRAM") as dr,
        ):
            t = sb.tile([128, 4], F32, tag="t", name="t")
            nc.sync.dma_start(t[:], x[:])
            bi = dr.tile([128, 4], F32, tag="bi", name="bi")
            bo = dr.tile([128, 4], F32, tag="bo", name="bo")
            nc.sync.dma_start(bi[:], t[:])
            nc.gpsimd.collective_compute(
                "AllReduce",
                mybir.AluOpType.add,
                replica_groups=[[0, 1, 2, 3], [4, 5, 6, 7]],
                ins=[bi.opt()],
                outs=[bo.opt()],
            )
            t2 = sb.tile([128, 4], F32, tag="t2", name="t2")
            nc.sync.dma_start(t2[:], bo[:])
            nc.sync.dma_start(y[:], t2[:])
    nc.finalize()
    z = np.zeros((128, 4), np.float32)
    run_bass_kernel_spmd(nc, [{"x": z}] * NCORES, core_ids=list(range(NCORES)))
    _warmed_up = True


def _interleave(a):
    """[D, X] -> [KK, 128, 2, X] DoubleRow layout: (kk, p, i, x) reads
    contraction row 256*kk + 128*i + p."""
    X = a.shape[1]
    return np.ascontiguousarray(
        a.reshape(KK, 2, 128, X).transpose(0, 2, 1, 3)
    )


def kernel(hidden, copy_attn, src_map, W, b, w_copy, b_copy, _trace=False):
    hidden = np.asarray(hidden, np.float32)
    copy_attn = np.asarray(copy_attn, np.float32)
    src_map = np.asarray(src_map, np.float32)
    W = np.asarray(W, np.float32)
    b = np.asarray(b, np.float32)
    w_copy = np.asarray(w_copy, np.float32)
    b_copy_f = float(np.asarray(b_copy))

    with_bias = bool(np.any(b != 0.0))
    pad_corr = float(np.exp(b[PAD])) if with_bias else 1.0

    F8NP = ml_dtypes.float8_e4m3

    # host-side shard prep (layout only; W[PAD,:] is dead data in the ref)
    Wz = W.copy()
    Wz[PAD, :] = 0.0
    WT8 = (Wz.T * WSC).astype(F8NP)                                  # [D, V]
    hT_f = np.ascontiguousarray(hidden.T)                            # [D, R] f32
    hT8 = hT_f.astype(F8NP)
    hT_b = hT_f.astype(ml_dtypes.bfloat16)
    hT_lo = (hT_f - hT_b.astype(np.float32)).astype(ml_dtypes.bfloat16)
    wc32 = w_copy.reshape(D, 1).astype(np.float32)
    wc_hi = wc32.astype(ml_dtypes.bfloat16)
    wc_lo = (wc32 - wc_hi.astype(np.float32)).astype(ml_dtypes.bfloat16)
    wc8_h = (wc32 * WSC).astype(F8NP)
    attnT_full = np.ascontiguousarray(copy_attn.T).astype(np.float16)  # [S, R]
    smap16 = src_map.astype(np.float16)                              # [B,S,CV]

    _warmup_collectives()
    nc = build_program(with_bias, b_copy_f, pad_corr)

    in_maps = []
    for c in range(NCORES):
        h, q = divmod(c, NQ)
        rows = slice(h * RH, (h + 1) * RH)
        crows = slice(c * LB * RB, (c + 1) * LB * RB)
        m = {
            "hT2": _interleave(hT8[:, rows]),
            "wT2": _interleave(WT8[:, q * VS:(q + 1) * VS]),
            "wc8": wc8_h,
            "hTlh": np.ascontiguousarray(hT_b[:, crows]),
            "hTll": np.ascontiguousarray(hT_lo[:, crows]),
            "wch": wc_hi,
            "wcl": wc_lo,
            "attnT": np.ascontiguousarray(attnT_full[:, crows]),
            "smap": np.ascontiguousarray(smap16[c * LB:(c + 1) * LB]),
        }
        if with_bias:
            eb = np.exp(b[q * VS:(q + 1) * VS].astype(np.float64)).astype(
                np.float32
            )
            m["ebb"] = np.ascontiguousarray(
                np.broadcast_to(eb[None, :], (128, VS))
            )
        in_maps.append(m)

    trace_cores = None
    if os.environ.get("TRACE_ALL_CORES"):
        trace_cores = list(range(NCORES))
    res = run_bass_kernel_spmd(
        nc, in_maps, core_ids=list(range(NCORES)), trace=_trace,
        trace_cores=trace_cores,
    )

    out = np.empty((R, V + CV), np.float32)
    for c in range(NCORES):
        h, q = divmod(c, NQ)
        out[h * RH:(h + 1) * RH, q * VS:(q + 1) * VS] = (
            res.results[c]["og"].astype(np.float32)
        )
        out[c * LB * RB:(c + 1) * LB * RB, V:] = res.results[c]["oc"]
    out[:, PAD] = 0.0

    if _trace:
        kernel.last_results = res
    return out


kernel.last_results = None


# revision 21
# speedup vs baseline: 1.0230x; 1.0230x over previous
"""Bass/Trainium2 kernel for nn_CopyGenerator (8-core SPMD).

Sharding: 4-way vocab (tensor parallel) x 2-way rows (data parallel).
Core c = 4*h + q owns rows [2048h, 2048h+2048) and vocab columns
[8000q, 8000q+8000).  The softmax denominator needs a cross-vocab-shard
sum: one AllReduce over 4 ranks per tapered group of row-blocks (GS),
in two independent replica groups ([[0,1,2,3],[4,5,6,7]]) that pipeline
behind compute.  The copy branch stays batch-sharded 8 ways.  A tiny
warmup NEFF with one AllReduce runs first (first collective after boot
pays ~60-75us of channel start latency).

Main-loop engine budget per core (16 row-blocks x 8000 vocab):
  - PE: fp8e4 DoubleRow matmuls (2 fp8 weights/cell, K=256 per pass).
    W is scaled by 32 on host so its values sit in fp8e4's normal range;
    the 1/32 rides the ACT exp scale.  Per 2048-col PSUM chunk: kk0
    stationary -> 4 N<=512 MMs (start), kk1 stationary -> 4 MMs (stop).
    The 256-col LDWEIGHTS (~215ns) hides behind 241ns MMs via the
    background weight buffer.  ~123us PE vs ~266us for bf16.
  - ACT: Exp over [128,2048] PSUM chunks with accum_out row sums
    (~2.0us each, ~128us total) - the critical engine.
  - DVE: per-block scale by m=(1-gate)/S into f16 staging (~40us).
  - DMA: og stores f16 (32MB/core, ~92us spread over rings).
PSUM: one pool of 2x[128,2048]f32 (8 banks); the gate and copy-branch
matmuls borrow slices of the same rotating tiles before the main loop.

Numerics: fp8 h/W give ~1.2% relative error on gen-branch probs; the
output absmax lives in the (untouched, fp16-exact) copy branch, so the
global rel-err stays ~3e-4.  f16 og storage adds only 0.05% (vs 0.4%
bf16).  PAD: host zeroes W[PAD,:], exp(0)=1 is subtracted from the
reduced sum (pad_corr), host zeroes output column PAD.
"""

import os
import sys

for _p in ("/opt/trn_rl_repo", "/root/.axon_site/_ro/trn_rl_repo"):
    if os.path.isdir(_p) and _p not in sys.path:
        sys.path.insert(0, _p)

import numpy as np
import ml_dtypes

import concourse.bacc as bacc
import concourse.tile as tile
from concourse import mybir
from concourse.bass_utils import run_bass_kernel_spmd

# ---------------------------------------------------------------------------
# Problem dimensions (hardcoded per spec)
# ---------------------------------------------------------------------------
B, T, S, V, CV, D = 32, 128, 400, 32000, 600, 512
PAD = 1
NCORES = 8
NQ = 4                    # vocab shards
NH = 2                    # row halves
R = B * T                 # 4096 rows
VS = V // NQ              # 8000 vocab columns per core
RH = R // NH              # 2048 rows per core
RB = 128                  # rows per block (= one batch: T == 128)
NBL = RH // RB            # 16 row blocks per core
# tapered all-reduce groups: the first ARs are delayed ~80us by per-NEFF
# collective-channel init anyway, so front groups are big; the last groups
# are size-1 to shrink the AR+scale+store drain tail
GS = [3, 3, 3, 3, 2, 2]   # sums to NBL
NG = len(GS)
GOFF = [sum(GS[:i]) for i in range(NG)]
GRPOF = []                # block -> (group, index-in-group)
for _g, _n in enumerate(GS):
    for _j in range(_n):
        GRPOF.append((_g, _j))
LB = B // NCORES          # 4 local batches per core (copy branch)
KK = 2                    # 256-deep DoubleRow contraction chunks (2x256=512)
WSC = 32.0                # host-side W scale (fp8 subnormal dodge)
# vocab chunking within a block (PSUM chunk: [128,2048]f32 = 4 banks)
VCH = [2048, 2048, 2048, 1856]  # = 8000
VOFF = [0, 2048, 4096, 6144]
NVC = len(VCH)
# s-dim chunks for the copy branch: 400 = 128+128+128+16
SCH = [128, 128, 128, 16]
SOFF = [0, 128, 256, 384]

F32 = mybir.dt.float32
F16 = mybir.dt.float16
BF16 = mybir.dt.bfloat16
F8 = mybir.dt.float8e4
DR = mybir.MatmulPerfMode.DoubleRow

EXP_BUFS = 28   # in-flight exp tiles ([128,2048] f16) = 7 blocks
OUT_BUFS = 2    # [128, 4000] f16 output staging tiles (2 per block)


def _mm_splits(n):
    """Split a free-dim span into <=512 pieces aligned to 512 (PSUM banks)."""
    out = []
    off = 0
    while off < n:
        w = min(512, n - off)
        out.append((off, w))
        off += w
    return out


def build_program(with_bias: bool, b_copy: float, pad_corr: float):
    # Bacc (not plain Bass): its finalize() runs move_matmul_waits_to_ldweights
    # + generate_event_semaphores, which split multi-sem waits down to the
    # TRN2 limit of one wait per instruction.
    nc = bacc.Bacc()

    hT2 = nc.dram_tensor("hT2", [KK, 128, 2, RH], F8, kind="ExternalInput")
    wT2 = nc.dram_tensor("wT2", [KK, 128, 2, VS], F8, kind="ExternalInput")
    wc8 = nc.dram_tensor("wc8", [D, 1], F8, kind="ExternalInput")
    hTlh = nc.dram_tensor("hTlh", [D, LB * RB], BF16, kind="ExternalInput")
    hTll = nc.dram_tensor("hTll", [D, LB * RB], BF16, kind="ExternalInput")
    wch = nc.dram_tensor("wch", [D, 1], BF16, kind="ExternalInput")
    wcl = nc.dram_tensor("wcl", [D, 1], BF16, kind="ExternalInput")
    attnT = nc.dram_tensor("attnT", [S, LB * RB], F16, kind="ExternalInput")
    smap = nc.dram_tensor("smap", [LB, S, CV], F16, kind="ExternalInput")
    if with_bias:
        ebb = nc.dram_tensor("ebb", [128, VS], F32, kind="ExternalInput")

    og = nc.dram_tensor("og", [RH, VS], F16, kind="ExternalOutput")
    oc = nc.dram_tensor("oc", [LB * RB, CV], F32, kind="ExternalOutput")

    with tile.TileContext(nc) as tc:
        with (
            tc.tile_pool(name="const", bufs=1) as const,
            tc.tile_pool(name="pm", bufs=2, space="PSUM") as pm,
            tc.tile_pool(name="expp", bufs=EXP_BUFS) as expp,
            tc.tile_pool(name="outp", bufs=OUT_BUFS) as outp,
            tc.tile_pool(name="ocp", bufs=2) as ocp,
            tc.tile_pool(name="smapp", bufs=LB * 4) as smapp,
            tc.tile_pool(name="small", bufs=10) as small,
            tc.tile_pool(name="gatep", bufs=NBL + LB) as gatep,
            tc.tile_pool(name="dram", bufs=1, space="DRAM") as dram,
        ):
            # ---------------- prologue: resident loads ----------------
            hT2_t = []
            wT2_t = []
            hTl_t = []
            wc_t = []
            wc8_t = []
            attnT_t = []
            ebb_t = []
            # small copy-branch inputs on the sync ring; the copy branch is
            # emitted before the main loop so PE has work during weight loads
            for k in range(D // 128):
                th = const.tile([128, LB * RB], BF16, tag=f"hTlh{k}", name=f"hTlh{k}")
                nc.sync.dma_start(th[:], hTlh[k * 128:(k + 1) * 128, :])
                tl = const.tile([128, LB * RB], BF16, tag=f"hTll{k}", name=f"hTll{k}")
                nc.sync.dma_start(tl[:], hTll[k * 128:(k + 1) * 128, :])
                hTl_t.append((th, tl))
                th = const.tile([128, 1], BF16, tag=f"wch{k}", name=f"wch{k}")
                nc.sync.dma_start(th[:], wch[k * 128:(k + 1) * 128, :])
                tl = const.tile([128, 1], BF16, tag=f"wcl{k}", name=f"wcl{k}")
                nc.sync.dma_start(tl[:], wcl[k * 128:(k + 1) * 128, :])
                wc_t.append((th, tl))
                t8 = const.tile([128, 1], F8, tag=f"wc8{k}", name=f"wc8{k}")
                nc.sync.dma_start(t8[:], wc8[k * 128:(k + 1) * 128, :])
                wc8_t.append(t8)
            for k in range(4):
                sk = SCH[k]
                t = const.tile([128, LB * RB], F16, tag=f"attnT{k}", name=f"attnT{k}")
                nc.sync.dma_start(t[:sk, :], attnT[SOFF[k]:SOFF[k] + sk, :])
                attnT_t.append(t)
            # prefetch ALL smap tiles on the sync ring: streamed on the
            # scalar ring they queue behind the 4MB wT2 shard, and the
            # copy-branch psum tiles that wait on them gate the whole pm-pool
            # rotation (blocks included) until ~30us
            sm_t = [[None] * 4 for _ in range(LB)]
            for l in range(LB):
                for k in range(4):
                    sk = SCH[k]
                    sm = smapp.tile([128, CV], F16, tag="sm", name="sm")
                    nc.sync.dma_start(
                        sm[:sk, :], smap[l, SOFF[k]:SOFF[k] + sk, :]
                    )
                    sm_t[l][k] = sm
            # big resident weights: hT2 on the gpsimd ring, wT2 on the ACT
            # ring, ordered so early blocks' dependencies land first
            for kk in range(KK):
                t = const.tile([128, 2, RH], F8, tag=f"hT2{kk}", name=f"hT2{kk}")
                hT2_t.append(t)
                t = const.tile([128, 2, VS], F8, tag=f"wT2{kk}", name=f"wT2{kk}")
                wT2_t.append(t)
            for kk in range(KK):
                nc.gpsimd.dma_start(hT2_t[kk][:], hT2[kk])
            for i in range(NVC):
                vo, n = VOFF[i], VCH[i]
                for kk in range(KK):
                    nc.scalar.dma_start(
                        wT2_t[kk][:, :, vo:vo + n], wT2[kk, :, :, vo:vo + n]
                    )
            if with_bias:
                for i in range(NVC):
                    t = const.tile([128, VCH[i]], F32, tag=f"ebb{i}", name=f"ebb{i}")
                    nc.sync.dma_start(t[:], ebb[:, VOFF[i]:VOFF[i] + VCH[i]])
                    ebb_t.append(t)

            # ---------------- main loop ----------------
            exp_tiles = [[None] * NVC for _ in range(NBL)]
            e_tiles = [None] * NBL    # exp(-gate logit) per block [128,1] f32
            u_tiles = [None] * NBL    # 1 + e per block [128,1] f32
            sg_tiles = [None] * NG    # group local sums [128, GROUP]
            cc_out = [None] * NG      # group all-reduced sums (SBUF)

            def compute_gates_all():
                # all 16 gates into ONE psum tile (one column per block):
                # separate [128,1] psum tiles would ping-pong the 2-buf pool
                # against ACT with a ~1us sem round-trip per gate (~13us of
                # PE idle); here the 64 tiny MMs run back-to-back and a
                # single ACT exp covers every gate
                gps_all = pm.tile([128, 2048], F32, tag="pm", name="gps_all")
                for jb in range(NBL):
                    cb = slice(jb * RB, (jb + 1) * RB)
                    kc = 0
                    for kk in range(KK):
                        for i in range(2):
                            nc.tensor.matmul(
                                gps_all[:, jb:jb + 1],
                                hT2_t[kk][:, i, cb], wc8_t[kc][:],
                                start=(kc == 0), stop=(kc == 3),
                            )
                            kc += 1
                # sigmoid via the Exp table: e = exp(-(x/WSC + b_copy))
                e_all = gatep.tile([128, NBL], F32, tag="e", name="e", bufs=1)
                nc.scalar.activation(
                    e_all[:], gps_all[:, :NBL],
                    mybir.ActivationFunctionType.Exp,
                    bias=-float(b_copy), scale=-1.0 / WSC,
                )
                u_all = gatep.tile([128, NBL], F32, tag="u", name="u", bufs=1)
                nc.vector.tensor_scalar_add(u_all[:], e_all[:], 1.0)
                for jb in range(NBL):
                    e_tiles[jb] = e_all[:, jb:jb + 1]
                    u_tiles[jb] = u_all[:, jb:jb + 1]

            def compute_block(jb):
                cb = slice(jb * RB, (jb + 1) * RB)
                sp = small.tile([128, NVC], F32, tag="sp", name="sp")
                for i in range(NVC):
                    n = VCH[i]
                    ps = pm.tile([128, 2048], F32, tag="pm", name="pm")
                    for kk in range(KK):
                        for (o, w) in _mm_splits(n):
                            nc.tensor.matmul(
                                ps[:, o:o + w],
                                hT2_t[kk][:, :, cb],
                                wT2_t[kk][:, :, VOFF[i] + o:VOFF[i] + o + w],
                                start=(kk == 0), stop=(kk == KK - 1),
                                perf_mode=DR,
                            )
                    ex = expp.tile([128, 2048], F16, tag="exp", name="exp")
                    if not with_bias:
                        nc.scalar.activation(
                            ex[:, :n], ps[:, :n],
                            mybir.ActivationFunctionType.Exp,
                            scale=1.0 / WSC,
                            accum_out=sp[:, i:i + 1],
                        )
                    else:
                        nc.scalar.activation(
                            ex[:, :n], ps[:, :n],
                            mybir.ActivationFunctionType.Exp,
                            scale=1.0 / WSC,
                        )
                        nc.vector.tensor_tensor(
                            ex[:, :n], ex[:, :n], ebb_t[i][:, :n],
                            mybir.AluOpType.mult,
                        )
                        nc.vector.reduce_sum(
                            sp[:, i:i + 1], ex[:, :n],
                            axis=mybir.AxisListType.X,
                        )
                    exp_tiles[jb][i] = ex
                g, j = GRPOF[jb]
                nc.vector.reduce_sum(
                    sg_tiles[g][:, j:j + 1], sp[:], axis=mybir.AxisListType.X
                )

            def scale_block(jb):
                g, j = GRPOF[jb]
                sgl = cc_out[g]
                # m = (1-gate)/S = e / ((1+e) * (S_allreduce - pad_corr))
                corr = small.tile([128, 1], F32, tag="corr", name="corr")
                nc.vector.tensor_scalar_add(corr[:], sgl[:, j:j + 1], -pad_corr)
                v = small.tile([128, 1], F32, tag="v", name="v")
                nc.vector.tensor_scalar(
                    v[:], corr[:], u_tiles[jb][:], None, mybir.AluOpType.mult
                )
                rec = small.tile([128, 1], F32, tag="rec", name="rec")
                nc.vector.reciprocal(rec[:], v[:])
                m = small.tile([128, 1], F32, tag="m", name="m")
                nc.vector.tensor_scalar(
                    m[:], rec[:], e_tiles[jb][:], None, mybir.AluOpType.mult
                )
                # scale exp chunks into f16 staging tiles, 2 stores per block
                for half in range(2):
                    hn = 4096 if half == 0 else VS - 4096
                    ot = outp.tile([128, 4096], F16, tag="ot", name="ot")
                    for i in range(2 * half, 2 * half + 2):
                        n = VCH[i]
                        oo = VOFF[i] - 4096 * half
                        nc.vector.tensor_scalar(
                            ot[:, oo:oo + n],
                            exp_tiles[jb][i][:, :n], m[:], None,
                            mybir.AluOpType.mult,
                        )
                        exp_tiles[jb][i] = None
                    nc.sync.dma_start(
                        og[jb * RB:(jb + 1) * RB, 4096 * half:4096 * half + hn],
                        ot[:, :hn],
                    )

            # ---------------- copy branch (batch-sharded) ----------------
            def emit_copy_branch():
                # emitted first: no dependence on collectives or the big weights
                # local gates: fp32-grade dot via bf16 hi/lo split, all LB
                # gates into one psum tile (see compute_gates_all)
                gpsc = pm.tile([128, 2048], F32, tag="pm", name="gpsc")
                nmm = 3 * (D // 128)
                for l in range(LB):
                    tb = slice(l * RB, (l + 1) * RB)
                    imm = 0
                    for k in range(D // 128):
                        for (a, b_) in ((0, 0), (0, 1), (1, 0)):
                            nc.tensor.matmul(
                                gpsc[:, l:l + 1], hTl_t[k][a][:, tb],
                                wc_t[k][b_][:],
                                start=(imm == 0), stop=(imm == nmm - 1),
                            )
                            imm += 1
                el = gatep.tile([128, LB], F32, tag="el", name="el", bufs=1)
                nc.scalar.activation(
                    el[:], gpsc[:, :LB], mybir.ActivationFunctionType.Exp,
                    bias=-float(b_copy), scale=-1.0,
                )
                ul = gatep.tile([128, LB], F32, tag="ul", name="ul", bufs=1)
                nc.vector.tensor_scalar_add(ul[:], el[:], 1.0)
                gl = gatep.tile([128, LB], F32, tag="gl", name="gl", bufs=1)
                nc.vector.reciprocal(gl[:], ul[:])
                for l in range(LB):
                    tb = slice(l * RB, (l + 1) * RB)
                    cpsf = pm.tile([128, 2048], F32, tag="pm", name="cpsf")
                    cps = cpsf[:, :CV]
                    for k in range(4):
                        sk = SCH[k]
                        sm = sm_t[l][k]
                        for (o, w) in _mm_splits(CV):
                            nc.tensor.matmul(
                                cps[:, o:o + w],
                                attnT_t[k][:sk, tb],
                                sm[:sk, o:o + w],
                                start=(k == 0), stop=(k == 3),
                            )
                    oct_ = ocp.tile([128, CV], F32, tag="oct", name="oct")
                    nc.vector.tensor_scalar(
                        oct_[:], cps, gl[:, l:l + 1], None, mybir.AluOpType.mult
                    )
                    nc.sync.dma_start(oc[tb, :], oct_[:])

            # dummy AllReduce issued early: the collective channel pays
            # ~65us of per-NEFF init on its first use, which would otherwise
            # delay group 0's real AllReduce (and stall the exp-tile pool)
            dz = small.tile([128, 1], F32, tag="dz", name="dz")
            nc.vector.memset(dz[:], 0.0)
            din = dram.tile([128, 1], F32, tag="din", name="din")
            dout = dram.tile([128, 1], F32, tag="dout", name="dout")
            nc.gpsimd.dma_start(din[:], dz[:])
            nc.gpsimd.collective_compute(
                "AllReduce",
                mybir.AluOpType.add,
                replica_groups=[[0, 1, 2, 3], [4, 5, 6, 7]],
                ins=[din.opt()],
                outs=[dout.opt()],
            )

            emit_copy_branch()
            # all 16 gates up-front: they only need hT2 + wc8 (small, fast
            # loads), so they fill the PE while the 4 MB wT2 shard streams in
            compute_gates_all()

            for g in range(NG):
                gn = GS[g]
                sg_tiles[g] = small.tile([128, gn], F32, tag="sg", name="sg")
                for j in range(gn):
                    compute_block(GOFF[g] + j)
                # all-reduce this group's local sums across the 4 vocab shards
                cin = dram.tile([128, gn], F32, tag=f"cin{g}", name=f"cin{g}")
                cout = dram.tile([128, gn], F32, tag=f"cout{g}", name=f"cout{g}")
                nc.gpsimd.dma_start(cin[:], sg_tiles[g][:])
                nc.gpsimd.collective_compute(
                    "AllReduce",
                    mybir.AluOpType.add,
                    replica_groups=[[0, 1, 2, 3], [4, 5, 6, 7]],
                    ins=[cin.opt()],
                    outs=[cout.opt()],
                )
                sgl = small.tile([128, gn], F32, tag="sgl", name="sgl")
                nc.gpsimd.dma_start(sgl[:], cout[:])
                cc_out[g] = sgl
                for j in range(gn):
                    scale_block(GOFF[g] + j)

    nc.finalize()
    return nc


_warmed_up = False


def _warmup_collectives():
    """Run a minimal NEFF with one AllReduce so the collective channel
    (ncfw firmware / TOPSP) is warm before the main kernel executes."""
    global _warmed_up
    if _warmed_up:
        return
    nc = bacc.Bacc()
    x = nc.dram_tensor("x", [128, 4], F32, kind="ExternalInput")
    y = nc.dram_tensor("y", [128, 4], F32, kind="ExternalOutput")
    with tile.TileContext(nc) as tc:
        with (
            tc.tile_pool(name="sb", bufs=2) as sb,
            tc.tile_pool(name="dr", bufs=2, space="DRAM") as dr,
        ):
            t = sb.tile([128, 4], F32, tag="t", name="t")
            nc.sync.dma_start(t[:], x[:])
            bi = dr.tile([128, 4], F32, tag="bi", name="bi")
            bo = dr.tile([128, 4], F32, tag="bo", name="bo")
            nc.sync.dma_start(bi[:], t[:])
            nc.gpsimd.collective_compute(
                "AllReduce",
                mybir.AluOpType.add,
                replica_groups=[[0, 1, 2, 3], [4, 5, 6, 7]],
                ins=[bi.opt()],
                outs=[bo.opt()],
            )
            t2 = sb.tile([128, 4], F32, tag="t2", name="t2")
            nc.sync.dma_start(t2[:], bo[:])
            nc.sync.dma_start(y[:], t2[:])
    nc.finalize()
    z = np.zeros((128, 4), np.float32)
    run_bass_kernel_spmd(nc, [{"x": z}] * NCORES, core_ids=list(range(NCORES)))
    _warmed_up = True


def _interleave(a):
    """[D, X] -> [KK, 128, 2, X] DoubleRow layout: (kk, p, i, x) reads
    contraction row 256*kk + 128*i + p."""
    X = a.shape[1]
    return np.ascontiguousarray(
        a.reshape(KK, 2, 128, X).transpose(0, 2, 1, 3)
    )


def kernel(hidden, copy_attn, src_map, W, b, w_copy, b_copy, _trace=False):
    hidden = np.asarray(hidden, np.float32)
    copy_attn = np.asarray(copy_attn, np.float32)
    src_map = np.asarray(src_map, np.float32)
    W = np.asarray(W, np.float32)
    b = np.asarray(b, np.float32)
    w_copy = np.asarray(w_copy, np.float32)
    b_copy_f = float(np.asarray(b_copy))

    with_bias = bool(np.any(b != 0.0))
    pad_corr = float(np.exp(b[PAD])) if with_bias else 1.0

    F8NP = ml_dtypes.float8_e4m3

    # host-side shard prep (layout only; W[PAD,:] is dead data in the ref)
    Wz = W.copy()
    Wz[PAD, :] = 0.0
    WT8 = (Wz.T * WSC).astype(F8NP)                                  # [D, V]
    hT_f = np.ascontiguousarray(hidden.T)                            # [D, R] f32
    hT8 = hT_f.astype(F8NP)
    hT_b = hT_f.astype(ml_dtypes.bfloat16)
    hT_lo = (hT_f - hT_b.astype(np.float32)).astype(ml_dtypes.bfloat16)
    wc32 = w_copy.reshape(D, 1).astype(np.float32)
    wc_hi = wc32.astype(ml_dtypes.bfloat16)
    wc_lo = (wc32 - wc_hi.astype(np.float32)).astype(ml_dtypes.bfloat16)
    wc8_h = (wc32 * WSC).astype(F8NP)
    attnT_full = np.ascontiguousarray(copy_attn.T).astype(np.float16)  # [S, R]
    smap16 = src_map.astype(np.float16)                              # [B,S,CV]

    _warmup_collectives()
    nc = build_program(with_bias, b_copy_f, pad_corr)

    in_maps = []
    for c in range(NCORES):
        h, q = divmod(c, NQ)
        rows = slice(h * RH, (h + 1) * RH)
        crows = slice(c * LB * RB, (c + 1) * LB * RB)
        m = {
            "hT2": _interleave(hT8[:, rows]),
            "wT2": _interleave(WT8[:, q * VS:(q + 1) * VS]),
            "wc8": wc8_h,
            "hTlh": np.ascontiguousarray(hT_b[:, crows]),
            "hTll": np.ascontiguousarray(hT_lo[:, crows]),
            "wch": wc_hi,
            "wcl": wc_lo,
            "attnT": np.ascontiguousarray(attnT_full[:, crows]),
            "smap": np.ascontiguousarray(smap16[c * LB:(c + 1) * LB]),
        }
        if with_bias:
            eb = np.exp(b[q * VS:(q + 1) * VS].astype(np.float64)).astype(
                np.float32
            )
            m["ebb"] = np.ascontiguousarray(
                np.broadcast_to(eb[None, :], (128, VS))
            )
        in_maps.append(m)

    trace_cores = None
    if os.environ.get("TRACE_ALL_CORES"):
        trace_cores = list(range(NCORES))
    res = run_bass_kernel_spmd(
        nc, in_maps, core_ids=list(range(NCORES)), trace=_trace,
        trace_cores=trace_cores,
    )

    out = np.empty((R, V + CV), np.float32)
    for c in range(NCORES):
        h, q = divmod(c, NQ)
        out[h * RH:(h + 1) * RH, q * VS:(q + 1) * VS] = (
            res.results[c]["og"].astype(np.float32)
        )
        out[c * LB * RB:(c + 1) * LB * RB, V:] = res.results[c]["oc"]
    out[:, PAD] = 0.0

    if _trace:
        kernel.last_results = res
    return out


kernel.last_results = None
